# revision 14
# baseline (speedup 1.0000x reference)
"""Trainium2 Bass kernel for nn_DeformConv_23278722744918.

The reference passes raw integer pixel coordinates to grid_sample as if they
were normalized [-1,1] coords (align_corners=True). After de-normalization,
xpix = (clip(h+i,0,95)+1)*47.5 and ypix = (clip(w+j,0,95)+1)*47.5, so every
sample with h+i >= 2 or w+j >= 2 lands outside [0,95] and is zero
(padding_mode='zeros').  Only four tap values survive, shared by all (h,w):

  A = 0.25*(x[47,47]+x[47,48]+x[48,47]+x[48,48])   (coord cases 0,0)
  B = 0.50*(x[47,95]+x[48,95])                     (coord cases 1,0)
  C = 0.50*(x[95,47]+x[95,48])                     (coord cases 0,1)
  D =       x[95,95]                               (coord cases 1,1)

After the stride-3 VALID conv over the rearranged feature map, the output is
b_conv everywhere except the 2x2 corner (per batch, out-channel):

  out[b,o,0,0] = sum_c A*w00 + C*w01 + B*w10 + D*w11   (+ b_conv[o])
  out[b,o,0,1] = sum_c C*w00 + D*w10
  out[b,o,1,0] = sum_c B*w00 + D*w01
  out[b,o,1,1] = sum_c D*w00

(w_ij = w_conv[o,c,i,j]; the offset-conv branch is dead: + 0.0*sum(off).)

Sharding: output channels are split 8 ways across the NeuronCores (the batch
dim is only 4); the sampled rows of x (y=47,48,95) are replicated.  Each core
gathers its bilinear taps on-device (reduces + scales), runs the 4 corner
matmuls on the TensorEngine, fills its background tile and writes its
[4,8,96,96] output shard with disjoint DMAs spread over both HWDGE rings.

Two program variants: b_conv==0 (always true for this problem's
setup_inputs) uses a pure-memset background; nonzero b_conv broadcasts the
bias with a DVE copy (step-0 AP) instead.
"""

import numpy as np

B, IC, IH, IW = 4, 64, 96, 96
OC = 64
NCORES = 8
OCP = OC // NCORES  # out channels per core
HW = IH * IW        # 9216
QCH = HW // 4       # 2304: background tile free size (4 partition-chunks/plane)

_ROWS = (47, 48, 95)  # sampled rows of x (y coords); cols sampled: 47,48,95

_prog_cache = {}


def _build_program(with_bias):
    """One SPMD Bass program: identical on every core; per-core data differs."""
    import concourse.bacc as bacc
    import concourse.bass as bass
    import concourse.mybir as mybir
    import concourse.tile as tile

    nc = bacc.Bacc()
    dt = mybir.dt.float32

    xr_d = nc.declare_dram_parameter("xr", [IC, B, 3, IW], dt, isOutput=False)
    wb_d = nc.declare_dram_parameter("wb", [128, 64], dt, isOutput=False)
    if with_bias:
        b128_d = nc.declare_dram_parameter("bias128", [128, 1], dt, isOutput=False)
        bv_d = nc.declare_dram_parameter("biasV", [B, OCP, 2, 2], dt, isOutput=False)
    out_d = nc.declare_dram_parameter("out", [B, OCP, IH, IW], dt, isOutput=True)

    with tile.TileContext(nc) as tc:
        with (
            tc.tile_pool(name="sbuf", bufs=1) as pool,
            tc.tile_pool(name="psum", bufs=1, space=bass.MemorySpace.PSUM) as psum,
        ):
            xr = pool.tile([IC, B, 3, IW], dt)
            w2 = pool.tile([128, 64], dt)
            S3 = pool.tile([128, 4], dt)
            S4 = pool.tile([128, 4], dt)
            V = pool.tile([B, OCP, 2, 2], dt)
            bg = pool.tile([128, QCH], dt)
            Vp = psum.tile([B, 32], dt)

            # Background: bg[q*32 + b*8 + o, r] = b_conv[o] (or just zeros).
            # The fill gates the big output writes, so keep it off the Scalar
            # engine (ACT table load + drain costs ~6us) and split the plain
            # memset across DVE and GpSimd.
            if with_bias:
                b128 = pool.tile([128, 1], dt)
                bv = pool.tile([B, OCP, 2, 2], dt)
                nc.sync.dma_start(b128[:], b128_d[:])
                nc.sync.dma_start(bv[:], bv_d[:])
                nc.vector.tensor_copy(bg[:], b128[:, 0:1].to_broadcast((128, QCH)))
            else:
                nc.vector.memset(bg[:, 0 : QCH // 2], 0.0)
                nc.gpsimd.memset(bg[:, QCH // 2 : QCH], 0.0)

            nc.sync.dma_start(xr[:], xr_d[:])
            nc.sync.dma_start(w2[:], wb_d[:])

            # Bilinear tap sums (bilinear scale factors are folded into the
            # host-prepared weight matrix).  Taps stacked on the K axis:
            # S3 = [A (c rows 0:64) | B (rows 64:128)], S4 = [C | D],
            # free dim = b.  xr rows: 0->y47, 1->y48, 2->y95.
            AX = mybir.AxisListType
            nc.vector.reduce_sum(S3[0:64, :], xr[:, :, 0:2, 47:49], axis=AX.XY)
            nc.vector.reduce_sum(S3[64:128, :], xr[:, :, 0:2, 95:96], axis=AX.XY)
            nc.vector.reduce_sum(S4[0:64, :], xr[:, :, 2:3, 47:49], axis=AX.XY)
            nc.vector.tensor_copy(S4[64:128, :], xr[:, :, 2:3, 95:96])

            # Corner values Vp[b, (h*2+w)*8+o] in two K=128 matmuls; w2 holds
            # the scale-folded, zero-padded tap weights (cols 0:32 for A|B,
            # 32:64 for C|D).
            MM = nc.tensor.matmul
            MM(Vp[:], S3[:], w2[:, 0:32], start=True, stop=False)
            MM(Vp[:], S4[:], w2[:, 32:64], start=False, stop=True)

            # Permute Vp's (h*2+w)*8+o free layout to V's (o,h,w) so the
            # corner DMA's last dim (w) is contiguous in SBUF.
            nc.vector.tensor_copy(
                V[:].rearrange("b o h w -> b o (h w)"),
                Vp[:].rearrange("b (hw o) -> b o hw", o=OCP),
            )
            if with_bias:
                nc.vector.tensor_add(V[:], V[:], bv[:])

            # Output shard writes, all disjoint: plane hw = q*2304 + r,
            # corner positions hw in {0,1,96,97} come from V.  Alternate the
            # two HWDGE rings (sync=SP, scalar=ACT) for parallel issue.
            ov = out_d[:].rearrange("b o h w -> (b o) (h w)")
            nc.sync.dma_start(ov[:, 2:96], bg[0:32, 2:96])
            nc.scalar.dma_start(ov[:, 98:QCH], bg[0:32, 98:QCH])
            nc.sync.dma_start(ov[:, QCH : 2 * QCH], bg[32:64, :])
            nc.scalar.dma_start(ov[:, 2 * QCH : 3 * QCH], bg[64:96, :])
            nc.sync.dma_start(ov[:, 3 * QCH : 4 * QCH], bg[96:128, :])
            nc.sync.dma_start(out_d[:, :, 0:1, 0:2], V[:, :, 0:1, :])
            nc.scalar.dma_start(out_d[:, :, 1:2, 0:2], V[:, :, 1:2, :])

    nc.finalize()  # Bacc.finalize runs the wait-splitting legalization passes
    return nc


def _get_program(with_bias):
    key = bool(with_bias)
    if key not in _prog_cache:
        _prog_cache[key] = _build_program(key)
    return _prog_cache[key]


def _make_in_maps(x, w_conv, b_conv, with_bias=None):
    x = np.ascontiguousarray(x, dtype=np.float32)
    w_conv = np.ascontiguousarray(w_conv, dtype=np.float32)
    b_conv = np.ascontiguousarray(b_conv, dtype=np.float32)
    if with_bias is None:
        with_bias = bool(np.any(b_conv != 0))

    xr = np.ascontiguousarray(x[:, :, _ROWS, :].transpose(1, 0, 2, 3))
    in_maps = []
    for core in range(NCORES):
        o0 = core * OCP
        wsl = w_conv[o0 : o0 + OCP, :, 0:2, 0:2]  # [8,64,2,2] (o,c,i,j)
        # w_pq[pq][c, o] helper: w_conv[o,c,i,j] transposed to [c,o]
        wco = lambda i, j: wsl[:, :, i, j].T  # [64(c), 8(o)]
        z = np.zeros((IC, OCP), np.float32)
        # Vp[b, pq*8+o] = sum_k S[k,b]*w2[k, pq*8+o]; K rows: A=0:64, B=64:128
        # (first matmul), C=0:64, D=64:128 (second).  Scales folded here.
        wA = np.concatenate([0.25 * wco(0, 0), z, z, z], axis=1)          # out00
        wB = np.concatenate([0.5 * wco(1, 0), z, 0.5 * wco(0, 0), z], axis=1)
        wC = np.concatenate([0.5 * wco(0, 1), 0.5 * wco(0, 0), z, z], axis=1)
        wD = np.concatenate([wco(1, 1), wco(1, 0), wco(0, 1), wco(0, 0)], axis=1)
        w2 = np.ascontiguousarray(
            np.concatenate(
                [np.concatenate([wA, wB], axis=0), np.concatenate([wC, wD], axis=0)],
                axis=1,
            )
        )  # [128, 64]
        m = {"xr": xr, "wb": w2}
        if with_bias:
            bc8 = b_conv[o0 : o0 + OCP]
            m["bias128"] = np.ascontiguousarray(np.tile(bc8, 16)[:, None])
            # biasV[b, o, h, w] = bc8[o]
            m["biasV"] = np.ascontiguousarray(
                np.broadcast_to(np.repeat(bc8, 4)[None, :], (B, 32))
            ).reshape(B, OCP, 2, 2)
        in_maps.append(m)
    return in_maps


def _run(x, w_conv, b_conv, trace=False, **spmd_kwargs):
    from concourse.bass_utils import run_bass_kernel_spmd

    with_bias = bool(np.any(np.asarray(b_conv) != 0))
    nc = _get_program(with_bias)
    in_maps = _make_in_maps(x, w_conv, b_conv, with_bias)
    res = run_bass_kernel_spmd(
        nc, in_maps, core_ids=list(range(NCORES)), trace=trace, **spmd_kwargs
    )
    out = np.concatenate([r["out"] for r in res.results], axis=1)
    return out, res


def kernel(x, w_off, b_off, w_conv, b_conv):
    out, _ = _run(x, w_conv, b_conv, trace=False)
    return out
